# revision 27
# baseline (speedup 1.0000x reference)
"""3-layer GCN (DGL GraphConv norm='both') on 8 Trainium2 NeuronCores.

Distribution: nodes (and their dst-partitioned edges) sharded across the 8
cores; per layer the scaled feature table is AllGather'd (in 4 overlapping
chunks) so every core can gather arbitrary src rows; aggregation is done per
128-node dst group with one-hot matmuls accumulating in PSUM (the one-hot
selection blocks are host-precomputed constants streamed over the idle HWDGE
queue); the dense W matmul + bias (outer-product matmul into PSUM) +
deg_dst scaling + relu (+ next layer's deg_src pre-scaling) are fused per
group into a single vector op.

Host-side work is integer index preprocessing only (edge bucketing, permuted
gather indices, degree bincount, 0/1 selection masks); all floating-point
math on x/W runs on device.
"""

import os
import numpy as np

import concourse.bacc as bacc
import concourse.bass as bass
import concourse.tile as tile
from concourse import mybir
from concourse.bass_utils import run_bass_kernel_spmd

# problem shapes (hardcoded per harness contract)
N = 50000
E = 800000
D = 128
DOUT = 64
NC = 8
SHARD = N // NC            # 6250
NG = (SHARD + 127) // 128  # 49 groups of 128 dst nodes
GP = NG * 128              # 6272 padded shard rows
HALF_ROWS = 4 * GP         # 25088 (< 2**15, fits int16 gather indices)
SG_SIZE = 4                # dst groups per supergroup (PSUM residency)
CHUNK_BLK = 16             # 2048 indices per dma_gather call
N_QUEUES = 4               # SWDGE queues round-robined across gather calls
# AllGather half boundaries within a shard (group-aligned); each half is
# gathered into its own Shared table (8*4096=32768 / 8*2176=17408 rows,
# max index 32767 so gather indices fit int16); the small second half makes
# the end-of-layer AllGather tail short
AGH = [0, 4096, 6272]
HSIZE = [4096, 2176]

F32 = mybir.dt.float32
BF16 = mybir.dt.bfloat16
I16 = mybir.dt.int16
NPBF = mybir.dt.np(mybir.dt.bfloat16)

last_exec_time_ns = None


def _gather_idx(src):
    """(half, row) in the per-half Shared table for global node n:
    half h table = concat over ranks m of hs_m[AGH[h]:AGH[h+1]]."""
    m = src // SHARD
    loc = src % SHARD
    h = (loc >= AGH[1]).astype(np.int64)
    hbase = np.take(np.array(AGH[:2]), h)
    hsize = np.take(np.array(HSIZE), h)
    return h, m * hsize + (loc - hbase)


def _prep_edges(src, dst):
    """Bucket edges by (core, dst-group, table-half); build a core-uniform
    padded block structure plus per-core gather-index / one-hot arrays."""
    src = np.asarray(src).astype(np.int64)
    dst = np.asarray(dst).astype(np.int64)

    half, gidx_local = _gather_idx(src)

    core = dst // SHARD
    dloc = dst % SHARD
    dgrp = dloc // 128
    drel = (dloc % 128).astype(np.int64)

    key = (core * NG + dgrp) * 2 + half
    order = np.argsort(key, kind="stable")
    key_sorted = key[order]
    bounds = np.searchsorted(key_sorted, np.arange(NC * NG * 2 + 1))

    # common (max-over-cores) block counts per (group, half)
    nb = np.zeros((NG, 2), np.int64)
    for g in range(NG):
        for h in range(2):
            mx = 0
            for c in range(NC):
                k = (c * NG + g) * 2 + h
                mx = max(mx, bounds[k + 1] - bounds[k])
            nb[g, h] = -(-mx // 128)

    sgs = [list(range(s, min(s + SG_SIZE, NG))) for s in range(0, NG, SG_SIZE)]
    layout = []
    pos = 0
    for sg in sgs:
        entry = {}
        for h in range(2):
            es = pos
            glist = []
            off = 0
            for g in sg:
                if nb[g, h]:
                    glist.append((g, off, int(nb[g, h])))
                    off += int(nb[g, h])
            pos += off * 128
            entry[h] = (es, off, glist)
        layout.append((sg, entry))
    nidx = pos
    nblk_total = nidx // 128

    gidx_cores = []
    oh_cores = []
    for c in range(NC):
        gi = np.zeros(nidx, np.int16)
        dr = np.full(nidx, -1, np.int64)
        for sg, entry in layout:
            for h in range(2):
                es, nbk, glist = entry[h]
                for g, off, nbg in glist:
                    k = (c * NG + g) * 2 + h
                    ids = order[bounds[k]:bounds[k + 1]]
                    s = es + off * 128
                    gi[s:s + len(ids)] = gidx_local[ids].astype(np.int16)
                    dr[s:s + len(ids)] = drel[ids]
        gidx_cores.append(
            np.tile(np.ascontiguousarray(gi.reshape(-1, 16).T), (8, 1)))
        # one-hot constants [128, nblk, 128]: [p, b, j] = (dr[b*128+p] == j)
        oh = np.zeros((nblk_total * 128, 128), np.float32)
        valid = dr >= 0
        oh[np.nonzero(valid)[0], dr[valid]] = 1.0
        oh_cores.append(np.ascontiguousarray(
            oh.reshape(nblk_total, 128, 128).transpose(1, 0, 2)
            .reshape(128, nblk_total * 128)).astype(NPBF))
    return layout, nidx, nblk_total, gidx_cores, oh_cores


def _to_pgrid(arr_shard, fill=0.0):
    """[SHARD, k] row-major -> [128, NG*k] partition-grid layout."""
    k = arr_shard.shape[1] if arr_shard.ndim == 2 else 1
    a = arr_shard.reshape(SHARD, k).astype(np.float32)
    pad = np.full((GP, k), fill, np.float32)
    pad[:SHARD] = a
    return np.ascontiguousarray(
        pad.reshape(NG, 128, k).transpose(1, 0, 2).reshape(128, NG * k))


def _build(layout, nidx, nblk_total):
    nc = bacc.Bacc("TRN2", target_bir_lowering=False, debug=False,
                   enable_asserts=False, num_devices=NC,
                   num_swdge_queues=N_QUEUES)

    xs_in = nc.dram_tensor("xs_in", [128, GP], F32, kind="ExternalInput")
    gidx_in = nc.dram_tensor("gidx_in", [128, nidx // 16], I16,
                             kind="ExternalInput")
    oh_in = nc.dram_tensor("oh_in", [128, nidx], BF16, kind="ExternalInput")
    degs_in = nc.dram_tensor("degs_in", [128, NG], F32, kind="ExternalInput")
    degd_in = nc.dram_tensor("degd_in", [128, NG], F32, kind="ExternalInput")
    # sqrt(clip(deg_dst,1)) as a row vector [1, GP] (for the bias pre-scale)
    degdr_in = nc.dram_tensor("degdr_in", [1, GP], F32, kind="ExternalInput")
    w_ins = [nc.dram_tensor(f"w{i}_in", [128, d], F32, kind="ExternalInput")
             for i, d in ((1, D), (2, D), (3, DOUT))]
    b_ins = [nc.dram_tensor(f"b{i}_in", [1, d], F32, kind="ExternalInput")
             for i, d in ((1, D), (2, D), (3, DOUT))]
    out_t = nc.dram_tensor("out_t", [GP, DOUT], F32, kind="ExternalOutput")

    with tile.TileContext(nc) as tc:
        with (
            tc.tile_pool(name="dram", bufs=1, space="DRAM") as dram,
            tc.tile_pool(name="const", bufs=1) as cp,
            tc.tile_pool(name="gath", bufs=12) as gpool,
            tc.tile_pool(name="ohp", bufs=12) as ohp,
            tc.tile_pool(name="small", bufs=4) as sp,
            tc.tile_pool(name="psum", bufs=2, space="PSUM") as pp,
        ):
            # --- constants to SBUF ---
            gidx = cp.tile([128, nidx // 16], I16)
            nc.sync.dma_start(gidx[:], gidx_in[:])
            wts, bts = [], []
            for i, d in ((0, D), (1, D), (2, DOUT)):
                wt = cp.tile([128, d], F32, name=f"wt{i}")
                bt = cp.tile([1, d], F32, name=f"bt{i}")
                nc.sync.dma_start(wt[:], w_ins[i][:])
                nc.sync.dma_start(bt[:], b_ins[i][:])
                wts.append(wt)
                bts.append(bt)

            # rsqrt(clip(deg,1)) for src and dst: [128, NG] grids
            rs = []
            for i, din in enumerate((degs_in, degd_in)):
                dt_ = cp.tile([128, NG], F32, name=f"deg{i}")
                rc = cp.tile([128, NG], F32, name=f"rec{i}")
                rq = cp.tile([128, NG], F32, name=f"rs{i}")
                nc.sync.dma_start(dt_[:], din[:])
                nc.vector.tensor_scalar(out=dt_[:], in0=dt_[:], scalar1=1.0,
                                        scalar2=None, op0=mybir.AluOpType.max)
                nc.vector.reciprocal(rc[:], dt_[:])
                nc.scalar.activation(rq[:], rc[:],
                                     mybir.ActivationFunctionType.Sqrt)
                rs.append(rq)
            rs_src, rs_dst = rs
            # combined scale rs_dst*rs_src (layers 1-2 epilogue)
            rs_ds = cp.tile([128, NG], F32)
            nc.vector.tensor_tensor(out=rs_ds[:], in0=rs_dst[:],
                                    in1=rs_src[:], op=mybir.AluOpType.mult)
            # sqrt(clip(deg_dst,1)) row vector for the bias pre-scale
            sqd_row = cp.tile([1, GP], F32)
            nc.sync.dma_start(sqd_row[:], degdr_in[:])
            nc.scalar.activation(sqd_row[:], sqd_row[:],
                                 mybir.ActivationFunctionType.Sqrt)

            # --- DRAM: per-layer AG input shard + per-half Shared tables ---
            hs = [dram.tile([GP, D], BF16, name=f"hs{i}") for i in range(3)]
            tbl = [[dram.tile([NC * HSIZE[h], D], BF16, addr_space="Shared",
                              name=f"tbl{i}_{h}") for h in range(2)]
                   for i in range(3)]

            def ag_half(li, h):
                r0, r1 = AGH[h], AGH[h + 1]
                nc.gpsimd.collective_compute(
                    "AllGather", mybir.AluOpType.bypass,
                    replica_groups=[list(range(NC))],
                    ins=[hs[li][r0:r1, :]],
                    outs=[tbl[li][h].opt()],
                )

            # --- layer-1 table: scale x shard by rsqrt(deg_src) ---
            for c0 in range(0, NG, 4):
                cw = min(4, NG - c0)
                xt = sp.tile([128, cw * 128], F32, tag="xs")
                xo = sp.tile([128, cw * 128], BF16, tag="xo")
                nc.sync.dma_start(xt[:], xs_in[:, c0 * 128:(c0 + cw) * 128])
                for j in range(cw):
                    g = c0 + j
                    nc.vector.tensor_scalar(
                        out=xo[:, j * 128:(j + 1) * 128],
                        in0=xt[:, j * 128:(j + 1) * 128],
                        scalar1=rs_src[:, g:g + 1], scalar2=None,
                        op0=mybir.AluOpType.mult)
                nc.sync.dma_start(
                    hs[0][c0 * 128:(c0 + cw) * 128, :]
                    .rearrange("(g p) f -> p g f", p=128),
                    xo[:].rearrange("p (g f) -> p g f", f=128))
                if (c0 + cw) * 128 == AGH[1]:
                    ag_half(0, 0)
            ag_half(0, 1)

            # --- 3 layers ---
            for li in range(3):
                fout = DOUT if li == 2 else D
                wt, bt = wts[li], bts[li]
                qrr = [0]
                for sg, entry in layout:
                    gt = {}
                    ot = {}
                    for h in range(2):
                        es, nbk, glist = entry[h]
                        if nbk == 0:
                            continue
                        gch, och = [], []
                        for cb in range(0, nbk, CHUNK_BLK):
                            cw = min(CHUNK_BLK, nbk - cb)
                            ces = es + cb * 128
                            g3 = gpool.tile([128, cw * 128], BF16, tag="gath",
                                            name=f"g{li}_{sg[0]}_{h}_{cb}")
                            nc.gpsimd.dma_gather(
                                out_ap=g3[:].rearrange(
                                    "p (c e) -> p c e", e=128),
                                in_ap=tbl[li][h][:],
                                idxs_ap=gidx[:, ces // 16:ces // 16 + cw * 8],
                                num_idxs=cw * 128,
                                num_idxs_reg=cw * 128,
                                elem_size=128,
                                single_packet=False,
                                queue_num=qrr[0] % N_QUEUES,
                            )
                            qrr[0] += 1
                            o3 = ohp.tile([128, cw * 128], BF16, tag="ohc",
                                          name=f"o{li}_{sg[0]}_{h}_{cb}")
                            nc.scalar.dma_start(
                                o3[:], oh_in[:, ces:ces + cw * 128])
                            gch.append(g3)
                            och.append(o3)
                        gt[h] = gch
                        ot[h] = och
                    for g in sg:
                        blocks = []
                        for h in range(2):
                            es, nbk, glist = entry[h]
                            for gg, off, nbg in glist:
                                if gg == g:
                                    for k in range(nbg):
                                        blocks.append((h, off + k))
                        if not blocks:
                            continue
                        psg = pp.tile([128, 128], F32, tag="agg", bufs=5,
                                      space="PSUM", name=f"ps{li}_{g}")
                        for j, (h, k) in enumerate(blocks):
                            ck, sl = divmod(k, CHUNK_BLK)
                            nc.tensor.matmul(
                                out=psg[:],
                                lhsT=gt[h][ck][:, sl * 128:(sl + 1) * 128],
                                rhs=ot[h][ck][:, sl * 128:(sl + 1) * 128],
                                start=(j == 0),
                                stop=(j == len(blocks) - 1))
                        aggT = sp.tile([128, 128], F32, tag="aggT",
                                       name=f"at{li}_{g}")
                        nc.vector.tensor_copy(out=aggT[:], in_=psg[:])
                        # z = outer(sqrt(deg_dst), b) + aggT.T @ W  in PSUM
                        zps = pp.tile([128, fout], F32, tag="z", bufs=2,
                                      space="PSUM", name=f"zp{li}_{g}")
                        nc.tensor.matmul(
                            out=zps[:],
                            lhsT=sqd_row[:, g * 128:(g + 1) * 128],
                            rhs=bt[:], start=True, stop=False)
                        nc.tensor.matmul(out=zps[:], lhsT=aggT[:], rhs=wt[:],
                                         start=False, stop=True)
                        # out = relu(z * s) with s = rs_dst (*rs_src for l<2)
                        z1 = sp.tile([128, fout], BF16 if li < 2 else F32,
                                     tag="z1", name=f"z1_{li}_{g}")
                        sv = rs_ds if li < 2 else rs_dst
                        nc.vector.tensor_scalar(
                            out=z1[:], in0=zps[:],
                            scalar1=sv[:, g:g + 1], scalar2=0.0,
                            op0=mybir.AluOpType.mult,
                            op1=mybir.AluOpType.max)
                        if li < 2:
                            nc.sync.dma_start(
                                hs[li + 1][g * 128:(g + 1) * 128, :], z1[:])
                        else:
                            nc.sync.dma_start(
                                out_t[g * 128:(g + 1) * 128, :], z1[:])
                    # fire AG halves for the next layer as groups complete
                    if li < 2:
                        done = (sg[-1] + 1) * 128
                        if done == AGH[1]:
                            ag_half(li + 1, 0)
                        elif done == GP:
                            ag_half(li + 1, 1)

    nc.compile()
    return nc


_cache = {}


def kernel(x, src, dst, W1, b1, W2, b2, W3, b3):
    global last_exec_time_ns
    x = np.asarray(x, np.float32)
    src_i = np.asarray(src)
    dst_i = np.asarray(dst)

    ckh = hash((src_i.tobytes(), dst_i.tobytes()))
    if ckh not in _cache:
        layout, nidx, nblk_total, gidx_cores, oh_cores = _prep_edges(
            src_i, dst_i)
        nc = _build(layout, nidx, nblk_total)
        _cache[ckh] = (nc, gidx_cores, oh_cores)
    nc, gidx_cores, oh_cores = _cache[ckh]

    deg_src = np.bincount(src_i.astype(np.int64), minlength=N).astype(np.float32)
    deg_dst = np.bincount(dst_i.astype(np.int64), minlength=N).astype(np.float32)

    in_maps = []
    for c in range(NC):
        sl = slice(c * SHARD, (c + 1) * SHARD)
        dd = np.ones(GP, np.float32)
        dd[:SHARD] = np.maximum(deg_dst[sl], 1.0)
        in_maps.append({
            "xs_in": _to_pgrid(x[sl]),
            "gidx_in": gidx_cores[c],
            "oh_in": oh_cores[c],
            "degs_in": _to_pgrid(deg_src[sl, None], fill=1.0),
            "degd_in": _to_pgrid(deg_dst[sl, None], fill=1.0),
            "degdr_in": dd[None, :],
            "w1_in": np.asarray(W1, np.float32),
            "w2_in": np.asarray(W2, np.float32),
            "w3_in": np.asarray(W3, np.float32),
            "b1_in": np.asarray(b1, np.float32)[None, :],
            "b2_in": np.asarray(b2, np.float32)[None, :],
            "b3_in": np.asarray(b3, np.float32)[None, :],
        })

    trace = bool(int(os.environ.get("GCN_TRACE", "0")))
    res = run_bass_kernel_spmd(nc, in_maps, core_ids=list(range(NC)),
                               trace=trace)
    last_exec_time_ns = res.exec_time_ns

    out = np.empty((N, DOUT), np.float32)
    for c in range(NC):
        out[c * SHARD:(c + 1) * SHARD] = res.results[c]["out_t"][:SHARD]
    return out


# revision 28
# speedup vs baseline: 1.0575x; 1.0575x over previous
"""3-layer GCN (DGL GraphConv norm='both') on 8 Trainium2 NeuronCores.

Distribution: nodes (and their dst-partitioned edges) sharded across the 8
cores; per layer the scaled feature table is AllGather'd (in 4 overlapping
chunks) so every core can gather arbitrary src rows; aggregation is done per
128-node dst group with one-hot matmuls accumulating in PSUM (the one-hot
selection blocks are host-precomputed constants streamed over the idle HWDGE
queue); the dense W matmul + bias (outer-product matmul into PSUM) +
deg_dst scaling + relu (+ next layer's deg_src pre-scaling) are fused per
group into a single vector op.

Host-side work is integer index preprocessing only (edge bucketing, permuted
gather indices, degree bincount, 0/1 selection masks); all floating-point
math on x/W runs on device.
"""

import os
import numpy as np

import concourse.bacc as bacc
import concourse.bass as bass
import concourse.tile as tile
from concourse import mybir
from concourse.bass_utils import run_bass_kernel_spmd

# problem shapes (hardcoded per harness contract)
N = 50000
E = 800000
D = 128
DOUT = 64
NC = 8
SHARD = N // NC            # 6250
NG = (SHARD + 127) // 128  # 49 groups of 128 dst nodes
GP = NG * 128              # 6272 padded shard rows
HALF_ROWS = 4 * GP         # 25088 (< 2**15, fits int16 gather indices)
SG_SIZE = 4                # dst groups per supergroup (PSUM residency)
CHUNK_BLK = 16             # 2048 indices per dma_gather call
N_QUEUES = 4               # SWDGE queues round-robined across gather calls
# AllGather half boundaries within a shard (group-aligned); each half is
# gathered into its own Shared table (8*4096=32768 / 8*2176=17408 rows,
# max index 32767 so gather indices fit int16); the small second half makes
# the end-of-layer AllGather tail short
AGH = [0, 3072, 6272]
HSIZE = [3072, 3200]

F32 = mybir.dt.float32
BF16 = mybir.dt.bfloat16
I16 = mybir.dt.int16
NPBF = mybir.dt.np(mybir.dt.bfloat16)

last_exec_time_ns = None


def _gather_idx(src):
    """(half, row) in the per-half Shared table for global node n:
    half h table = concat over ranks m of hs_m[AGH[h]:AGH[h+1]]."""
    m = src // SHARD
    loc = src % SHARD
    h = (loc >= AGH[1]).astype(np.int64)
    hbase = np.take(np.array(AGH[:2]), h)
    hsize = np.take(np.array(HSIZE), h)
    return h, m * hsize + (loc - hbase)


def _prep_edges(src, dst):
    """Bucket edges by (core, dst-group, table-half); build a core-uniform
    padded block structure plus per-core gather-index / one-hot arrays."""
    src = np.asarray(src).astype(np.int64)
    dst = np.asarray(dst).astype(np.int64)

    half, gidx_local = _gather_idx(src)

    core = dst // SHARD
    dloc = dst % SHARD
    dgrp = dloc // 128
    drel = (dloc % 128).astype(np.int64)

    key = (core * NG + dgrp) * 2 + half
    order = np.argsort(key, kind="stable")
    key_sorted = key[order]
    bounds = np.searchsorted(key_sorted, np.arange(NC * NG * 2 + 1))

    # common (max-over-cores) block counts per (group, half)
    nb = np.zeros((NG, 2), np.int64)
    for g in range(NG):
        for h in range(2):
            mx = 0
            for c in range(NC):
                k = (c * NG + g) * 2 + h
                mx = max(mx, bounds[k + 1] - bounds[k])
            nb[g, h] = -(-mx // 128)

    sgs = [list(range(s, min(s + SG_SIZE, NG))) for s in range(0, NG, SG_SIZE)]
    layout = []
    pos = 0
    for sg in sgs:
        entry = {}
        for h in range(2):
            es = pos
            glist = []
            off = 0
            for g in sg:
                if nb[g, h]:
                    glist.append((g, off, int(nb[g, h])))
                    off += int(nb[g, h])
            pos += off * 128
            entry[h] = (es, off, glist)
        layout.append((sg, entry))
    nidx = pos
    nblk_total = nidx // 128

    gidx_cores = []
    oh_cores = []
    for c in range(NC):
        gi = np.zeros(nidx, np.int16)
        dr = np.full(nidx, -1, np.int64)
        for sg, entry in layout:
            for h in range(2):
                es, nbk, glist = entry[h]
                for g, off, nbg in glist:
                    k = (c * NG + g) * 2 + h
                    ids = order[bounds[k]:bounds[k + 1]]
                    s = es + off * 128
                    gi[s:s + len(ids)] = gidx_local[ids].astype(np.int16)
                    dr[s:s + len(ids)] = drel[ids]
        gidx_cores.append(
            np.tile(np.ascontiguousarray(gi.reshape(-1, 16).T), (8, 1)))
        # one-hot constants [128, nblk, 128]: [p, b, j] = (dr[b*128+p] == j)
        oh = np.zeros((nblk_total * 128, 128), np.float32)
        valid = dr >= 0
        oh[np.nonzero(valid)[0], dr[valid]] = 1.0
        oh_cores.append(np.ascontiguousarray(
            oh.reshape(nblk_total, 128, 128).transpose(1, 0, 2)
            .reshape(128, nblk_total * 128)).astype(NPBF))
    return layout, nidx, nblk_total, gidx_cores, oh_cores


def _to_pgrid(arr_shard, fill=0.0):
    """[SHARD, k] row-major -> [128, NG*k] partition-grid layout."""
    k = arr_shard.shape[1] if arr_shard.ndim == 2 else 1
    a = arr_shard.reshape(SHARD, k).astype(np.float32)
    pad = np.full((GP, k), fill, np.float32)
    pad[:SHARD] = a
    return np.ascontiguousarray(
        pad.reshape(NG, 128, k).transpose(1, 0, 2).reshape(128, NG * k))


def _build(layout, nidx, nblk_total):
    nc = bacc.Bacc("TRN2", target_bir_lowering=False, debug=False,
                   enable_asserts=False, num_devices=NC,
                   num_swdge_queues=N_QUEUES)

    xs_in = nc.dram_tensor("xs_in", [128, GP], F32, kind="ExternalInput")
    gidx_in = nc.dram_tensor("gidx_in", [128, nidx // 16], I16,
                             kind="ExternalInput")
    oh_in = nc.dram_tensor("oh_in", [128, nidx], BF16, kind="ExternalInput")
    degs_in = nc.dram_tensor("degs_in", [128, NG], F32, kind="ExternalInput")
    degd_in = nc.dram_tensor("degd_in", [128, NG], F32, kind="ExternalInput")
    # sqrt(clip(deg_dst,1)) as a row vector [1, GP] (for the bias pre-scale)
    degdr_in = nc.dram_tensor("degdr_in", [1, GP], F32, kind="ExternalInput")
    w_ins = [nc.dram_tensor(f"w{i}_in", [128, d], F32, kind="ExternalInput")
             for i, d in ((1, D), (2, D), (3, DOUT))]
    b_ins = [nc.dram_tensor(f"b{i}_in", [1, d], F32, kind="ExternalInput")
             for i, d in ((1, D), (2, D), (3, DOUT))]
    out_t = nc.dram_tensor("out_t", [GP, DOUT], F32, kind="ExternalOutput")

    with tile.TileContext(nc) as tc:
        with (
            tc.tile_pool(name="dram", bufs=1, space="DRAM") as dram,
            tc.tile_pool(name="const", bufs=1) as cp,
            tc.tile_pool(name="gath", bufs=12) as gpool,
            tc.tile_pool(name="ohp", bufs=12) as ohp,
            tc.tile_pool(name="small", bufs=4) as sp,
            tc.tile_pool(name="psum", bufs=2, space="PSUM") as pp,
        ):
            # --- constants to SBUF ---
            gidx = cp.tile([128, nidx // 16], I16)
            nc.sync.dma_start(gidx[:], gidx_in[:])
            wts, bts = [], []
            for i, d in ((0, D), (1, D), (2, DOUT)):
                wt = cp.tile([128, d], F32, name=f"wt{i}")
                bt = cp.tile([1, d], F32, name=f"bt{i}")
                nc.sync.dma_start(wt[:], w_ins[i][:])
                nc.sync.dma_start(bt[:], b_ins[i][:])
                wts.append(wt)
                bts.append(bt)

            # rsqrt(clip(deg,1)) for src and dst: [128, NG] grids
            rs = []
            for i, din in enumerate((degs_in, degd_in)):
                dt_ = cp.tile([128, NG], F32, name=f"deg{i}")
                rc = cp.tile([128, NG], F32, name=f"rec{i}")
                rq = cp.tile([128, NG], F32, name=f"rs{i}")
                nc.sync.dma_start(dt_[:], din[:])
                nc.vector.tensor_scalar(out=dt_[:], in0=dt_[:], scalar1=1.0,
                                        scalar2=None, op0=mybir.AluOpType.max)
                nc.vector.reciprocal(rc[:], dt_[:])
                nc.scalar.activation(rq[:], rc[:],
                                     mybir.ActivationFunctionType.Sqrt)
                rs.append(rq)
            rs_src, rs_dst = rs
            # combined scale rs_dst*rs_src (layers 1-2 epilogue)
            rs_ds = cp.tile([128, NG], F32)
            nc.vector.tensor_tensor(out=rs_ds[:], in0=rs_dst[:],
                                    in1=rs_src[:], op=mybir.AluOpType.mult)
            # sqrt(clip(deg_dst,1)) row vector for the bias pre-scale
            sqd_row = cp.tile([1, GP], F32)
            nc.sync.dma_start(sqd_row[:], degdr_in[:])
            nc.scalar.activation(sqd_row[:], sqd_row[:],
                                 mybir.ActivationFunctionType.Sqrt)

            # --- DRAM: per-layer AG input shard + per-half Shared tables ---
            hs = [dram.tile([GP, D], BF16, name=f"hs{i}") for i in range(3)]
            tbl = [[dram.tile([NC * HSIZE[h], D], BF16, addr_space="Shared",
                              name=f"tbl{i}_{h}") for h in range(2)]
                   for i in range(3)]

            def ag_half(li, h):
                r0, r1 = AGH[h], AGH[h + 1]
                nc.gpsimd.collective_compute(
                    "AllGather", mybir.AluOpType.bypass,
                    replica_groups=[list(range(NC))],
                    ins=[hs[li][r0:r1, :]],
                    outs=[tbl[li][h].opt()],
                )

            # --- layer-1 table: scale x shard by rsqrt(deg_src) ---
            for c0 in range(0, NG, 4):
                cw = min(4, NG - c0)
                xt = sp.tile([128, cw * 128], F32, tag="xs")
                xo = sp.tile([128, cw * 128], BF16, tag="xo")
                nc.sync.dma_start(xt[:], xs_in[:, c0 * 128:(c0 + cw) * 128])
                for j in range(cw):
                    g = c0 + j
                    nc.vector.tensor_scalar(
                        out=xo[:, j * 128:(j + 1) * 128],
                        in0=xt[:, j * 128:(j + 1) * 128],
                        scalar1=rs_src[:, g:g + 1], scalar2=None,
                        op0=mybir.AluOpType.mult)
                nc.sync.dma_start(
                    hs[0][c0 * 128:(c0 + cw) * 128, :]
                    .rearrange("(g p) f -> p g f", p=128),
                    xo[:].rearrange("p (g f) -> p g f", f=128))
                if (c0 + cw) * 128 == AGH[1]:
                    ag_half(0, 0)
            ag_half(0, 1)

            # --- 3 layers ---
            for li in range(3):
                fout = DOUT if li == 2 else D
                wt, bt = wts[li], bts[li]
                qrr = [0]
                for sg, entry in layout:
                    gt = {}
                    ot = {}
                    for h in range(2):
                        es, nbk, glist = entry[h]
                        if nbk == 0:
                            continue
                        gch, och = [], []
                        for cb in range(0, nbk, CHUNK_BLK):
                            cw = min(CHUNK_BLK, nbk - cb)
                            ces = es + cb * 128
                            g3 = gpool.tile([128, cw * 128], BF16, tag="gath",
                                            name=f"g{li}_{sg[0]}_{h}_{cb}")
                            nc.gpsimd.dma_gather(
                                out_ap=g3[:].rearrange(
                                    "p (c e) -> p c e", e=128),
                                in_ap=tbl[li][h][:],
                                idxs_ap=gidx[:, ces // 16:ces // 16 + cw * 8],
                                num_idxs=cw * 128,
                                num_idxs_reg=cw * 128,
                                elem_size=128,
                                single_packet=False,
                                queue_num=qrr[0] % N_QUEUES,
                            )
                            qrr[0] += 1
                            o3 = ohp.tile([128, cw * 128], BF16, tag="ohc",
                                          name=f"o{li}_{sg[0]}_{h}_{cb}")
                            nc.scalar.dma_start(
                                o3[:], oh_in[:, ces:ces + cw * 128])
                            gch.append(g3)
                            och.append(o3)
                        gt[h] = gch
                        ot[h] = och
                    for g in sg:
                        blocks = []
                        for h in range(2):
                            es, nbk, glist = entry[h]
                            for gg, off, nbg in glist:
                                if gg == g:
                                    for k in range(nbg):
                                        blocks.append((h, off + k))
                        if not blocks:
                            continue
                        psg = pp.tile([128, 128], F32, tag="agg", bufs=5,
                                      space="PSUM", name=f"ps{li}_{g}")
                        for j, (h, k) in enumerate(blocks):
                            ck, sl = divmod(k, CHUNK_BLK)
                            nc.tensor.matmul(
                                out=psg[:],
                                lhsT=gt[h][ck][:, sl * 128:(sl + 1) * 128],
                                rhs=ot[h][ck][:, sl * 128:(sl + 1) * 128],
                                start=(j == 0),
                                stop=(j == len(blocks) - 1))
                        aggT = sp.tile([128, 128], F32, tag="aggT",
                                       name=f"at{li}_{g}")
                        nc.vector.tensor_copy(out=aggT[:], in_=psg[:])
                        # z = outer(sqrt(deg_dst), b) + aggT.T @ W  in PSUM
                        zps = pp.tile([128, fout], F32, tag="z", bufs=2,
                                      space="PSUM", name=f"zp{li}_{g}")
                        nc.tensor.matmul(
                            out=zps[:],
                            lhsT=sqd_row[:, g * 128:(g + 1) * 128],
                            rhs=bt[:], start=True, stop=False)
                        nc.tensor.matmul(out=zps[:], lhsT=aggT[:], rhs=wt[:],
                                         start=False, stop=True)
                        # out = relu(z * s) with s = rs_dst (*rs_src for l<2)
                        z1 = sp.tile([128, fout], BF16 if li < 2 else F32,
                                     tag="z1", name=f"z1_{li}_{g}")
                        sv = rs_ds if li < 2 else rs_dst
                        nc.vector.tensor_scalar(
                            out=z1[:], in0=zps[:],
                            scalar1=sv[:, g:g + 1], scalar2=0.0,
                            op0=mybir.AluOpType.mult,
                            op1=mybir.AluOpType.max)
                        if li < 2:
                            nc.sync.dma_start(
                                hs[li + 1][g * 128:(g + 1) * 128, :], z1[:])
                        else:
                            nc.sync.dma_start(
                                out_t[g * 128:(g + 1) * 128, :], z1[:])
                    # fire AG halves for the next layer as groups complete
                    if li < 2:
                        done = (sg[-1] + 1) * 128
                        if done == AGH[1]:
                            ag_half(li + 1, 0)
                        elif done == GP:
                            ag_half(li + 1, 1)

    nc.compile()
    return nc


_cache = {}


def kernel(x, src, dst, W1, b1, W2, b2, W3, b3):
    global last_exec_time_ns
    x = np.asarray(x, np.float32)
    src_i = np.asarray(src)
    dst_i = np.asarray(dst)

    ckh = hash((src_i.tobytes(), dst_i.tobytes()))
    if ckh not in _cache:
        layout, nidx, nblk_total, gidx_cores, oh_cores = _prep_edges(
            src_i, dst_i)
        nc = _build(layout, nidx, nblk_total)
        _cache[ckh] = (nc, gidx_cores, oh_cores)
    nc, gidx_cores, oh_cores = _cache[ckh]

    deg_src = np.bincount(src_i.astype(np.int64), minlength=N).astype(np.float32)
    deg_dst = np.bincount(dst_i.astype(np.int64), minlength=N).astype(np.float32)

    in_maps = []
    for c in range(NC):
        sl = slice(c * SHARD, (c + 1) * SHARD)
        dd = np.ones(GP, np.float32)
        dd[:SHARD] = np.maximum(deg_dst[sl], 1.0)
        in_maps.append({
            "xs_in": _to_pgrid(x[sl]),
            "gidx_in": gidx_cores[c],
            "oh_in": oh_cores[c],
            "degs_in": _to_pgrid(deg_src[sl, None], fill=1.0),
            "degd_in": _to_pgrid(deg_dst[sl, None], fill=1.0),
            "degdr_in": dd[None, :],
            "w1_in": np.asarray(W1, np.float32),
            "w2_in": np.asarray(W2, np.float32),
            "w3_in": np.asarray(W3, np.float32),
            "b1_in": np.asarray(b1, np.float32)[None, :],
            "b2_in": np.asarray(b2, np.float32)[None, :],
            "b3_in": np.asarray(b3, np.float32)[None, :],
        })

    trace = bool(int(os.environ.get("GCN_TRACE", "0")))
    res = run_bass_kernel_spmd(nc, in_maps, core_ids=list(range(NC)),
                               trace=trace)
    last_exec_time_ns = res.exec_time_ns

    out = np.empty((N, DOUT), np.float32)
    for c in range(NC):
        out[c * SHARD:(c + 1) * SHARD] = res.results[c]["out_t"][:SHARD]
    return out


# revision 30
# speedup vs baseline: 1.0587x; 1.0011x over previous
"""3-layer GCN (DGL GraphConv norm='both') on 8 Trainium2 NeuronCores.

Distribution: nodes (and their dst-partitioned edges) sharded across the 8
cores; per layer the scaled feature table is AllGather'd (in 4 overlapping
chunks) so every core can gather arbitrary src rows; aggregation is done per
128-node dst group with one-hot matmuls accumulating in PSUM (the one-hot
selection blocks are host-precomputed constants streamed over the idle HWDGE
queue); the dense W matmul + bias (outer-product matmul into PSUM) +
deg_dst scaling + relu (+ next layer's deg_src pre-scaling) are fused per
group into a single vector op.

Host-side work is integer index preprocessing only (edge bucketing, permuted
gather indices, degree bincount, 0/1 selection masks); all floating-point
math on x/W runs on device.
"""

import os
import numpy as np

import concourse.bacc as bacc
import concourse.bass as bass
import concourse.tile as tile
from concourse import mybir
from concourse.bass_utils import run_bass_kernel_spmd

# problem shapes (hardcoded per harness contract)
N = 50000
E = 800000
D = 128
DOUT = 64
NC = 8
SHARD = N // NC            # 6250
NG = (SHARD + 127) // 128  # 49 groups of 128 dst nodes
GP = NG * 128              # 6272 padded shard rows
HALF_ROWS = 4 * GP         # 25088 (< 2**15, fits int16 gather indices)
SG_SIZE = 4                # dst groups per supergroup (PSUM residency)
CHUNK_BLK = 16             # 2048 indices per dma_gather call
N_QUEUES = 4               # SWDGE queues round-robined across gather calls
# AllGather half boundaries within a shard (group-aligned); each half is
# gathered into its own Shared table (8*4096=32768 / 8*2176=17408 rows,
# max index 32767 so gather indices fit int16); the small second half makes
# the end-of-layer AllGather tail short
AGH = [0, 3072, 6272]
HSIZE = [3072, 3200]

F32 = mybir.dt.float32
BF16 = mybir.dt.bfloat16
I16 = mybir.dt.int16
NPBF = mybir.dt.np(mybir.dt.bfloat16)

last_exec_time_ns = None


def _gather_idx(src):
    """(half, row) in the per-half Shared table for global node n:
    half h table = concat over ranks m of hs_m[AGH[h]:AGH[h+1]]."""
    m = src // SHARD
    loc = src % SHARD
    h = (loc >= AGH[1]).astype(np.int64)
    hbase = np.take(np.array(AGH[:2]), h)
    hsize = np.take(np.array(HSIZE), h)
    return h, m * hsize + (loc - hbase)


def _prep_edges(src, dst):
    """Bucket edges by (core, dst-group, table-half); build a core-uniform
    padded block structure plus per-core gather-index / one-hot arrays."""
    src = np.asarray(src).astype(np.int64)
    dst = np.asarray(dst).astype(np.int64)

    half, gidx_local = _gather_idx(src)

    core = dst // SHARD
    dloc = dst % SHARD
    dgrp = dloc // 128
    drel = (dloc % 128).astype(np.int64)

    key = (core * NG + dgrp) * 2 + half
    order = np.argsort(key, kind="stable")
    key_sorted = key[order]
    bounds = np.searchsorted(key_sorted, np.arange(NC * NG * 2 + 1))

    # common (max-over-cores) block counts per (group, half)
    nb = np.zeros((NG, 2), np.int64)
    for g in range(NG):
        for h in range(2):
            mx = 0
            for c in range(NC):
                k = (c * NG + g) * 2 + h
                mx = max(mx, bounds[k + 1] - bounds[k])
            nb[g, h] = -(-mx // 128)

    sgs = [list(range(s, min(s + SG_SIZE, NG))) for s in range(0, NG, SG_SIZE)]
    layout = []
    pos = 0
    for sg in sgs:
        entry = {}
        for h in range(2):
            es = pos
            glist = []
            off = 0
            for g in sg:
                if nb[g, h]:
                    glist.append((g, off, int(nb[g, h])))
                    off += int(nb[g, h])
            pos += off * 128
            entry[h] = (es, off, glist)
        layout.append((sg, entry))
    nidx = pos
    nblk_total = nidx // 128

    gidx_cores = []
    oh_cores = []
    for c in range(NC):
        gi = np.zeros(nidx, np.int16)
        dr = np.full(nidx, -1, np.int64)
        for sg, entry in layout:
            for h in range(2):
                es, nbk, glist = entry[h]
                for g, off, nbg in glist:
                    k = (c * NG + g) * 2 + h
                    ids = order[bounds[k]:bounds[k + 1]]
                    s = es + off * 128
                    gi[s:s + len(ids)] = gidx_local[ids].astype(np.int16)
                    dr[s:s + len(ids)] = drel[ids]
        gidx_cores.append(
            np.tile(np.ascontiguousarray(gi.reshape(-1, 16).T), (8, 1)))
        # one-hot constants [128, nblk, 128]: [p, b, j] = (dr[b*128+p] == j)
        oh = np.zeros((nblk_total * 128, 128), np.float32)
        valid = dr >= 0
        oh[np.nonzero(valid)[0], dr[valid]] = 1.0
        oh_cores.append(np.ascontiguousarray(
            oh.reshape(nblk_total, 128, 128).transpose(1, 0, 2)
            .reshape(128, nblk_total * 128)).astype(NPBF))
    return layout, nidx, nblk_total, gidx_cores, oh_cores


def _to_pgrid(arr_shard, fill=0.0):
    """[SHARD, k] row-major -> [128, NG*k] partition-grid layout."""
    k = arr_shard.shape[1] if arr_shard.ndim == 2 else 1
    a = arr_shard.reshape(SHARD, k).astype(np.float32)
    pad = np.full((GP, k), fill, np.float32)
    pad[:SHARD] = a
    return np.ascontiguousarray(
        pad.reshape(NG, 128, k).transpose(1, 0, 2).reshape(128, NG * k))


def _build(layout, nidx, nblk_total):
    nc = bacc.Bacc("TRN2", target_bir_lowering=False, debug=False,
                   enable_asserts=False, num_devices=NC,
                   num_swdge_queues=N_QUEUES)

    xs_in = nc.dram_tensor("xs_in", [128, GP], F32, kind="ExternalInput")
    gidx_in = nc.dram_tensor("gidx_in", [128, nidx // 16], I16,
                             kind="ExternalInput")
    oh_in = nc.dram_tensor("oh_in", [128, nidx], BF16, kind="ExternalInput")
    degs_in = nc.dram_tensor("degs_in", [128, NG], F32, kind="ExternalInput")
    degd_in = nc.dram_tensor("degd_in", [128, NG], F32, kind="ExternalInput")
    # sqrt(clip(deg_dst,1)) as a row vector [1, GP] (for the bias pre-scale)
    degdr_in = nc.dram_tensor("degdr_in", [1, GP], F32, kind="ExternalInput")
    w_ins = [nc.dram_tensor(f"w{i}_in", [128, d], F32, kind="ExternalInput")
             for i, d in ((1, D), (2, D), (3, DOUT))]
    b_ins = [nc.dram_tensor(f"b{i}_in", [1, d], F32, kind="ExternalInput")
             for i, d in ((1, D), (2, D), (3, DOUT))]
    out_t = nc.dram_tensor("out_t", [GP, DOUT], F32, kind="ExternalOutput")

    with tile.TileContext(nc) as tc:
        with (
            tc.tile_pool(name="dram", bufs=1, space="DRAM") as dram,
            tc.tile_pool(name="const", bufs=1) as cp,
            tc.tile_pool(name="gath", bufs=14) as gpool,
            tc.tile_pool(name="ohp", bufs=14) as ohp,
            tc.tile_pool(name="small", bufs=4) as sp,
            tc.tile_pool(name="psum", bufs=2, space="PSUM") as pp,
        ):
            # --- constants to SBUF ---
            gidx = cp.tile([128, nidx // 16], I16)
            nc.sync.dma_start(gidx[:], gidx_in[:])
            wts, bts = [], []
            for i, d in ((0, D), (1, D), (2, DOUT)):
                wt = cp.tile([128, d], F32, name=f"wt{i}")
                bt = cp.tile([1, d], F32, name=f"bt{i}")
                nc.sync.dma_start(wt[:], w_ins[i][:])
                nc.sync.dma_start(bt[:], b_ins[i][:])
                wts.append(wt)
                bts.append(bt)

            # rsqrt(clip(deg,1)) for src and dst: [128, NG] grids
            rs = []
            for i, din in enumerate((degs_in, degd_in)):
                dt_ = cp.tile([128, NG], F32, name=f"deg{i}")
                rc = cp.tile([128, NG], F32, name=f"rec{i}")
                rq = cp.tile([128, NG], F32, name=f"rs{i}")
                nc.sync.dma_start(dt_[:], din[:])
                nc.vector.tensor_scalar(out=dt_[:], in0=dt_[:], scalar1=1.0,
                                        scalar2=None, op0=mybir.AluOpType.max)
                nc.vector.reciprocal(rc[:], dt_[:])
                nc.scalar.activation(rq[:], rc[:],
                                     mybir.ActivationFunctionType.Sqrt)
                rs.append(rq)
            rs_src, rs_dst = rs
            # combined scale rs_dst*rs_src (layers 1-2 epilogue)
            rs_ds = cp.tile([128, NG], F32)
            nc.vector.tensor_tensor(out=rs_ds[:], in0=rs_dst[:],
                                    in1=rs_src[:], op=mybir.AluOpType.mult)
            # sqrt(clip(deg_dst,1)) row vector for the bias pre-scale
            sqd_row = cp.tile([1, GP], F32)
            nc.sync.dma_start(sqd_row[:], degdr_in[:])
            nc.scalar.activation(sqd_row[:], sqd_row[:],
                                 mybir.ActivationFunctionType.Sqrt)

            # --- DRAM: per-layer AG input shard + per-half Shared tables ---
            hs = [dram.tile([GP, D], BF16, name=f"hs{i}") for i in range(3)]
            tbl = [[dram.tile([NC * HSIZE[h], D], BF16, addr_space="Shared",
                              name=f"tbl{i}_{h}") for h in range(2)]
                   for i in range(3)]

            def ag_half(li, h):
                r0, r1 = AGH[h], AGH[h + 1]
                nc.gpsimd.collective_compute(
                    "AllGather", mybir.AluOpType.bypass,
                    replica_groups=[list(range(NC))],
                    ins=[hs[li][r0:r1, :]],
                    outs=[tbl[li][h].opt()],
                )

            # --- layer-1 table: scale x shard by rsqrt(deg_src) ---
            for c0 in range(0, NG, 4):
                cw = min(4, NG - c0)
                xt = sp.tile([128, cw * 128], F32, tag="xs")
                xo = sp.tile([128, cw * 128], BF16, tag="xo")
                nc.sync.dma_start(xt[:], xs_in[:, c0 * 128:(c0 + cw) * 128])
                for j in range(cw):
                    g = c0 + j
                    nc.vector.tensor_scalar(
                        out=xo[:, j * 128:(j + 1) * 128],
                        in0=xt[:, j * 128:(j + 1) * 128],
                        scalar1=rs_src[:, g:g + 1], scalar2=None,
                        op0=mybir.AluOpType.mult)
                nc.sync.dma_start(
                    hs[0][c0 * 128:(c0 + cw) * 128, :]
                    .rearrange("(g p) f -> p g f", p=128),
                    xo[:].rearrange("p (g f) -> p g f", f=128))
                if (c0 + cw) * 128 == AGH[1]:
                    ag_half(0, 0)
            ag_half(0, 1)

            # --- 3 layers ---
            for li in range(3):
                fout = DOUT if li == 2 else D
                wt, bt = wts[li], bts[li]
                qrr = [0]
                tiles = {}  # (sg_idx, h) -> (gather chunk tiles, oh tiles)

                def issue(si, h):
                    if (si, h) in tiles or si >= len(layout):
                        return
                    sg, entry = layout[si]
                    es, nbk, glist = entry[h]
                    if nbk == 0:
                        tiles[(si, h)] = ([], [])
                        return
                    gch, och = [], []
                    for cb in range(0, nbk, CHUNK_BLK):
                        cw = min(CHUNK_BLK, nbk - cb)
                        ces = es + cb * 128
                        g3 = gpool.tile([128, cw * 128], BF16, tag="gath",
                                        name=f"g{li}_{sg[0]}_{h}_{cb}")
                        nc.gpsimd.dma_gather(
                            out_ap=g3[:].rearrange("p (c e) -> p c e", e=128),
                            in_ap=tbl[li][h][:],
                            idxs_ap=gidx[:, ces // 16:ces // 16 + cw * 8],
                            num_idxs=cw * 128,
                            num_idxs_reg=cw * 128,
                            elem_size=128,
                            single_packet=False,
                            queue_num=qrr[0] % N_QUEUES,
                        )
                        qrr[0] += 1
                        o3 = ohp.tile([128, cw * 128], BF16, tag="ohc",
                                      name=f"o{li}_{sg[0]}_{h}_{cb}")
                        nc.scalar.dma_start(
                            o3[:], oh_in[:, ces:ces + cw * 128])
                        gch.append(g3)
                        och.append(o3)
                    tiles[(si, h)] = (gch, och)

                for si, (sg, entry) in enumerate(layout):
                    # skewed issue: half-0 gathers run 2 supergroups ahead
                    # so POOL has runway while the half-1 AllGather lands
                    issue(si, 0)
                    issue(si + 1, 0)
                    issue(si + 2, 0)
                    issue(si, 1)
                    gt = {h: tiles[(si, h)][0] for h in range(2)}
                    ot = {h: tiles[(si, h)][1] for h in range(2)}
                    for g in sg:
                        blocks = []
                        for h in range(2):
                            es, nbk, glist = entry[h]
                            for gg, off, nbg in glist:
                                if gg == g:
                                    for k in range(nbg):
                                        blocks.append((h, off + k))
                        if not blocks:
                            continue
                        psg = pp.tile([128, 128], F32, tag="agg", bufs=5,
                                      space="PSUM", name=f"ps{li}_{g}")
                        for j, (h, k) in enumerate(blocks):
                            ck, sl = divmod(k, CHUNK_BLK)
                            nc.tensor.matmul(
                                out=psg[:],
                                lhsT=gt[h][ck][:, sl * 128:(sl + 1) * 128],
                                rhs=ot[h][ck][:, sl * 128:(sl + 1) * 128],
                                start=(j == 0),
                                stop=(j == len(blocks) - 1))
                        aggT = sp.tile([128, 128], F32, tag="aggT",
                                       name=f"at{li}_{g}")
                        nc.vector.tensor_copy(out=aggT[:], in_=psg[:])
                        # z = outer(sqrt(deg_dst), b) + aggT.T @ W  in PSUM
                        zps = pp.tile([128, fout], F32, tag="z", bufs=2,
                                      space="PSUM", name=f"zp{li}_{g}")
                        nc.tensor.matmul(
                            out=zps[:],
                            lhsT=sqd_row[:, g * 128:(g + 1) * 128],
                            rhs=bt[:], start=True, stop=False)
                        nc.tensor.matmul(out=zps[:], lhsT=aggT[:], rhs=wt[:],
                                         start=False, stop=True)
                        # out = relu(z * s) with s = rs_dst (*rs_src for l<2)
                        z1 = sp.tile([128, fout], BF16 if li < 2 else F32,
                                     tag="z1", name=f"z1_{li}_{g}")
                        sv = rs_ds if li < 2 else rs_dst
                        nc.vector.tensor_scalar(
                            out=z1[:], in0=zps[:],
                            scalar1=sv[:, g:g + 1], scalar2=0.0,
                            op0=mybir.AluOpType.mult,
                            op1=mybir.AluOpType.max)
                        if li < 2:
                            nc.sync.dma_start(
                                hs[li + 1][g * 128:(g + 1) * 128, :], z1[:])
                        else:
                            nc.sync.dma_start(
                                out_t[g * 128:(g + 1) * 128, :], z1[:])
                    # fire AG halves for the next layer as groups complete
                    if li < 2:
                        done = (sg[-1] + 1) * 128
                        if done == AGH[1]:
                            ag_half(li + 1, 0)
                        elif done == GP:
                            ag_half(li + 1, 1)

    nc.compile()
    return nc


_cache = {}


def kernel(x, src, dst, W1, b1, W2, b2, W3, b3):
    global last_exec_time_ns
    x = np.asarray(x, np.float32)
    src_i = np.asarray(src)
    dst_i = np.asarray(dst)

    ckh = hash((src_i.tobytes(), dst_i.tobytes()))
    if ckh not in _cache:
        layout, nidx, nblk_total, gidx_cores, oh_cores = _prep_edges(
            src_i, dst_i)
        nc = _build(layout, nidx, nblk_total)
        _cache[ckh] = (nc, gidx_cores, oh_cores)
    nc, gidx_cores, oh_cores = _cache[ckh]

    deg_src = np.bincount(src_i.astype(np.int64), minlength=N).astype(np.float32)
    deg_dst = np.bincount(dst_i.astype(np.int64), minlength=N).astype(np.float32)

    in_maps = []
    for c in range(NC):
        sl = slice(c * SHARD, (c + 1) * SHARD)
        dd = np.ones(GP, np.float32)
        dd[:SHARD] = np.maximum(deg_dst[sl], 1.0)
        in_maps.append({
            "xs_in": _to_pgrid(x[sl]),
            "gidx_in": gidx_cores[c],
            "oh_in": oh_cores[c],
            "degs_in": _to_pgrid(deg_src[sl, None], fill=1.0),
            "degd_in": _to_pgrid(deg_dst[sl, None], fill=1.0),
            "degdr_in": dd[None, :],
            "w1_in": np.asarray(W1, np.float32),
            "w2_in": np.asarray(W2, np.float32),
            "w3_in": np.asarray(W3, np.float32),
            "b1_in": np.asarray(b1, np.float32)[None, :],
            "b2_in": np.asarray(b2, np.float32)[None, :],
            "b3_in": np.asarray(b3, np.float32)[None, :],
        })

    trace = bool(int(os.environ.get("GCN_TRACE", "0")))
    res = run_bass_kernel_spmd(nc, in_maps, core_ids=list(range(NC)),
                               trace=trace)
    last_exec_time_ns = res.exec_time_ns

    out = np.empty((N, DOUT), np.float32)
    for c in range(NC):
        out[c * SHARD:(c + 1) * SHARD] = res.results[c]["out_t"][:SHARD]
    return out


# revision 31
# speedup vs baseline: 1.0683x; 1.0091x over previous
"""3-layer GCN (DGL GraphConv norm='both') on 8 Trainium2 NeuronCores.

Distribution: nodes (and their dst-partitioned edges) sharded across the 8
cores; per layer the scaled feature table is AllGather'd (in 4 overlapping
chunks) so every core can gather arbitrary src rows; aggregation is done per
128-node dst group with one-hot matmuls accumulating in PSUM (the one-hot
selection blocks are host-precomputed constants streamed over the idle HWDGE
queue); the dense W matmul + bias (outer-product matmul into PSUM) +
deg_dst scaling + relu (+ next layer's deg_src pre-scaling) are fused per
group into a single vector op.

Host-side work is integer index preprocessing only (edge bucketing, permuted
gather indices, degree bincount, 0/1 selection masks); all floating-point
math on x/W runs on device.
"""

import os
import numpy as np

import concourse.bacc as bacc
import concourse.bass as bass
import concourse.tile as tile
from concourse import mybir
from concourse.bass_utils import run_bass_kernel_spmd

# problem shapes (hardcoded per harness contract)
N = 50000
E = 800000
D = 128
DOUT = 64
NC = 8
SHARD = N // NC            # 6250
NG = (SHARD + 127) // 128  # 49 groups of 128 dst nodes
GP = NG * 128              # 6272 padded shard rows
HALF_ROWS = 4 * GP         # 25088 (< 2**15, fits int16 gather indices)
SG_SIZE = 4                # dst groups per supergroup (PSUM residency)
CHUNK_BLK = 16             # 2048 indices per dma_gather call
N_QUEUES = 4               # SWDGE queues round-robined across gather calls
# AllGather half boundaries within a shard (group-aligned); each half is
# gathered into its own Shared table (8*4096=32768 / 8*2176=17408 rows,
# max index 32767 so gather indices fit int16); the small second half makes
# the end-of-layer AllGather tail short
AGH = [0, 3072, 6272]
HSIZE = [3072, 3200]

F32 = mybir.dt.float32
BF16 = mybir.dt.bfloat16
I16 = mybir.dt.int16
NPBF = mybir.dt.np(mybir.dt.bfloat16)

last_exec_time_ns = None


def _gather_idx(src):
    """(half, row) in the per-half Shared table for global node n:
    half h table = concat over ranks m of hs_m[AGH[h]:AGH[h+1]]."""
    m = src // SHARD
    loc = src % SHARD
    h = (loc >= AGH[1]).astype(np.int64)
    hbase = np.take(np.array(AGH[:2]), h)
    hsize = np.take(np.array(HSIZE), h)
    return h, m * hsize + (loc - hbase)


def _prep_edges(src, dst):
    """Bucket edges by (core, dst-group, table-half); build a core-uniform
    padded block structure plus per-core gather-index / one-hot arrays."""
    src = np.asarray(src).astype(np.int64)
    dst = np.asarray(dst).astype(np.int64)

    half, gidx_local = _gather_idx(src)

    core = dst // SHARD
    dloc = dst % SHARD
    dgrp = dloc // 128
    drel = (dloc % 128).astype(np.int64)

    key = (core * NG + dgrp) * 2 + half
    order = np.argsort(key, kind="stable")
    key_sorted = key[order]
    bounds = np.searchsorted(key_sorted, np.arange(NC * NG * 2 + 1))

    # common (max-over-cores) block counts per (group, half)
    nb = np.zeros((NG, 2), np.int64)
    for g in range(NG):
        for h in range(2):
            mx = 0
            for c in range(NC):
                k = (c * NG + g) * 2 + h
                mx = max(mx, bounds[k + 1] - bounds[k])
            nb[g, h] = -(-mx // 128)

    sgs = [list(range(s, min(s + SG_SIZE, NG))) for s in range(0, NG, SG_SIZE)]
    layout = []
    pos = 0
    for sg in sgs:
        entry = {}
        for h in range(2):
            es = pos
            glist = []
            off = 0
            for g in sg:
                if nb[g, h]:
                    glist.append((g, off, int(nb[g, h])))
                    off += int(nb[g, h])
            pos += off * 128
            entry[h] = (es, off, glist)
        layout.append((sg, entry))
    nidx = pos
    nblk_total = nidx // 128

    gidx_cores = []
    oh_cores = []
    for c in range(NC):
        gi = np.zeros(nidx, np.int16)
        dr = np.full(nidx, -1, np.int64)
        for sg, entry in layout:
            for h in range(2):
                es, nbk, glist = entry[h]
                for g, off, nbg in glist:
                    k = (c * NG + g) * 2 + h
                    ids = order[bounds[k]:bounds[k + 1]]
                    s = es + off * 128
                    gi[s:s + len(ids)] = gidx_local[ids].astype(np.int16)
                    dr[s:s + len(ids)] = drel[ids]
        gidx_cores.append(
            np.tile(np.ascontiguousarray(gi.reshape(-1, 16).T), (8, 1)))
        # one-hot constants [128, nblk, 128]: [p, b, j] = (dr[b*128+p] == j)
        oh = np.zeros((nblk_total * 128, 128), np.float32)
        valid = dr >= 0
        oh[np.nonzero(valid)[0], dr[valid]] = 1.0
        oh_cores.append(np.ascontiguousarray(
            oh.reshape(nblk_total, 128, 128).transpose(1, 0, 2)
            .reshape(128, nblk_total * 128)).astype(NPBF))
    return layout, nidx, nblk_total, gidx_cores, oh_cores


def _to_pgrid(arr_shard, fill=0.0):
    """[SHARD, k] row-major -> [128, NG*k] partition-grid layout."""
    k = arr_shard.shape[1] if arr_shard.ndim == 2 else 1
    a = arr_shard.reshape(SHARD, k).astype(np.float32)
    pad = np.full((GP, k), fill, np.float32)
    pad[:SHARD] = a
    return np.ascontiguousarray(
        pad.reshape(NG, 128, k).transpose(1, 0, 2).reshape(128, NG * k))


def _build(layout, nidx, nblk_total):
    nc = bacc.Bacc("TRN2", target_bir_lowering=False, debug=False,
                   enable_asserts=False, num_devices=NC,
                   num_swdge_queues=N_QUEUES)

    xs_in = nc.dram_tensor("xs_in", [128, GP], F32, kind="ExternalInput")
    gidx_in = nc.dram_tensor("gidx_in", [128, nidx // 16], I16,
                             kind="ExternalInput")
    oh_in = nc.dram_tensor("oh_in", [128, nidx], BF16, kind="ExternalInput")
    degs_in = nc.dram_tensor("degs_in", [128, NG], F32, kind="ExternalInput")
    degd_in = nc.dram_tensor("degd_in", [128, NG], F32, kind="ExternalInput")
    # sqrt(clip(deg_dst,1)) as a row vector [1, GP] (for the bias pre-scale)
    degdr_in = nc.dram_tensor("degdr_in", [1, GP], F32, kind="ExternalInput")
    w_ins = [nc.dram_tensor(f"w{i}_in", [128, d], F32, kind="ExternalInput")
             for i, d in ((1, D), (2, D), (3, DOUT))]
    b_ins = [nc.dram_tensor(f"b{i}_in", [1, d], F32, kind="ExternalInput")
             for i, d in ((1, D), (2, D), (3, DOUT))]
    out_t = nc.dram_tensor("out_t", [GP, DOUT], F32, kind="ExternalOutput")

    with tile.TileContext(nc) as tc:
        with (
            tc.tile_pool(name="dram", bufs=1, space="DRAM") as dram,
            tc.tile_pool(name="const", bufs=1) as cp,
            tc.tile_pool(name="gath", bufs=14) as gpool,
            tc.tile_pool(name="ohp", bufs=14) as ohp,
            tc.tile_pool(name="small", bufs=4) as sp,
            tc.tile_pool(name="psum", bufs=2, space="PSUM") as pp,
        ):
            # --- constants to SBUF ---
            gidx = cp.tile([128, nidx // 16], I16)
            nc.sync.dma_start(gidx[:], gidx_in[:])
            wts, bts = [], []
            for i, d in ((0, D), (1, D), (2, DOUT)):
                wt = cp.tile([128, d], F32, name=f"wt{i}")
                bt = cp.tile([1, d], F32, name=f"bt{i}")
                nc.sync.dma_start(wt[:], w_ins[i][:])
                nc.sync.dma_start(bt[:], b_ins[i][:])
                wts.append(wt)
                bts.append(bt)

            # rsqrt(clip(deg,1)) for src and dst: [128, NG] grids
            rs = []
            for i, din in enumerate((degs_in, degd_in)):
                dt_ = cp.tile([128, NG], F32, name=f"deg{i}")
                rc = cp.tile([128, NG], F32, name=f"rec{i}")
                rq = cp.tile([128, NG], F32, name=f"rs{i}")
                nc.sync.dma_start(dt_[:], din[:])
                nc.vector.tensor_scalar(out=dt_[:], in0=dt_[:], scalar1=1.0,
                                        scalar2=None, op0=mybir.AluOpType.max)
                nc.vector.reciprocal(rc[:], dt_[:])
                nc.scalar.activation(rq[:], rc[:],
                                     mybir.ActivationFunctionType.Sqrt)
                rs.append(rq)
            rs_src, rs_dst = rs
            # combined scale rs_dst*rs_src (layers 1-2 epilogue)
            rs_ds = cp.tile([128, NG], F32)
            nc.vector.tensor_tensor(out=rs_ds[:], in0=rs_dst[:],
                                    in1=rs_src[:], op=mybir.AluOpType.mult)
            # sqrt(clip(deg_dst,1)) row vector for the bias pre-scale
            sqd_row = cp.tile([1, GP], F32)
            nc.sync.dma_start(sqd_row[:], degdr_in[:])
            nc.scalar.activation(sqd_row[:], sqd_row[:],
                                 mybir.ActivationFunctionType.Sqrt)

            # --- DRAM: per-layer AG input shard + per-half Shared tables ---
            hs = [dram.tile([GP, D], BF16, name=f"hs{i}") for i in range(3)]
            tbl = [[dram.tile([NC * HSIZE[h], D], BF16, addr_space="Shared",
                              name=f"tbl{i}_{h}") for h in range(2)]
                   for i in range(3)]

            def ag_half(li, h):
                r0, r1 = AGH[h], AGH[h + 1]
                nc.gpsimd.collective_compute(
                    "AllGather", mybir.AluOpType.bypass,
                    replica_groups=[list(range(NC))],
                    ins=[hs[li][r0:r1, :]],
                    outs=[tbl[li][h].opt()],
                )

            # --- layer-1 table: scale x shard by rsqrt(deg_src) ---
            for c0 in range(0, NG, 4):
                cw = min(4, NG - c0)
                xt = sp.tile([128, cw * 128], F32, tag="xs")
                xo = sp.tile([128, cw * 128], BF16, tag="xo")
                nc.sync.dma_start(xt[:], xs_in[:, c0 * 128:(c0 + cw) * 128])
                for j in range(cw):
                    g = c0 + j
                    nc.vector.tensor_scalar(
                        out=xo[:, j * 128:(j + 1) * 128],
                        in0=xt[:, j * 128:(j + 1) * 128],
                        scalar1=rs_src[:, g:g + 1], scalar2=None,
                        op0=mybir.AluOpType.mult)
                nc.sync.dma_start(
                    hs[0][c0 * 128:(c0 + cw) * 128, :]
                    .rearrange("(g p) f -> p g f", p=128),
                    xo[:].rearrange("p (g f) -> p g f", f=128))
                if (c0 + cw) * 128 == AGH[1]:
                    ag_half(0, 0)
            ag_half(0, 1)

            # --- 3 layers ---
            for li in range(3):
                fout = DOUT if li == 2 else D
                wt, bt = wts[li], bts[li]
                qrr = [0]
                tiles = {}  # (sg_idx, h) -> (gather chunk tiles, oh tiles)

                def issue(si, h):
                    if (si, h) in tiles or si >= len(layout):
                        return
                    sg, entry = layout[si]
                    es, nbk, glist = entry[h]
                    if nbk == 0:
                        tiles[(si, h)] = ([], [])
                        return
                    gch, och = [], []
                    for cb in range(0, nbk, CHUNK_BLK):
                        cw = min(CHUNK_BLK, nbk - cb)
                        ces = es + cb * 128
                        g3 = gpool.tile([128, cw * 128], BF16, tag="gath",
                                        name=f"g{li}_{sg[0]}_{h}_{cb}")
                        nc.gpsimd.dma_gather(
                            out_ap=g3[:].rearrange("p (c e) -> p c e", e=128),
                            in_ap=tbl[li][h][:],
                            idxs_ap=gidx[:, ces // 16:ces // 16 + cw * 8],
                            num_idxs=cw * 128,
                            num_idxs_reg=cw * 128,
                            elem_size=128,
                            single_packet=False,
                            queue_num=qrr[0] % N_QUEUES,
                        )
                        qrr[0] += 1
                        o3 = ohp.tile([128, cw * 128], BF16, tag="ohc",
                                      name=f"o{li}_{sg[0]}_{h}_{cb}")
                        nc.scalar.dma_start(
                            o3[:], oh_in[:, ces:ces + cw * 128])
                        gch.append(g3)
                        och.append(o3)
                    tiles[(si, h)] = (gch, och)

                for si, (sg, entry) in enumerate(layout):
                    # issue half-0 one supergroup ahead so POOL has runway
                    # while the half-1 AllGather of the next table lands
                    issue(si, 0)
                    issue(si + 1, 0)
                    issue(si, 1)
                    gt = {h: tiles[(si, h)][0] for h in range(2)}
                    ot = {h: tiles[(si, h)][1] for h in range(2)}
                    for g in sg:
                        blocks = []
                        for h in range(2):
                            es, nbk, glist = entry[h]
                            for gg, off, nbg in glist:
                                if gg == g:
                                    for k in range(nbg):
                                        blocks.append((h, off + k))
                        if not blocks:
                            continue
                        psg = pp.tile([128, 128], F32, tag="agg", bufs=5,
                                      space="PSUM", name=f"ps{li}_{g}")
                        for j, (h, k) in enumerate(blocks):
                            ck, sl = divmod(k, CHUNK_BLK)
                            nc.tensor.matmul(
                                out=psg[:],
                                lhsT=gt[h][ck][:, sl * 128:(sl + 1) * 128],
                                rhs=ot[h][ck][:, sl * 128:(sl + 1) * 128],
                                start=(j == 0),
                                stop=(j == len(blocks) - 1))
                        aggT = sp.tile([128, 128], F32, tag="aggT",
                                       name=f"at{li}_{g}")
                        nc.vector.tensor_copy(out=aggT[:], in_=psg[:])
                        # z = outer(sqrt(deg_dst), b) + aggT.T @ W  in PSUM
                        zps = pp.tile([128, fout], F32, tag="z", bufs=2,
                                      space="PSUM", name=f"zp{li}_{g}")
                        nc.tensor.matmul(
                            out=zps[:],
                            lhsT=sqd_row[:, g * 128:(g + 1) * 128],
                            rhs=bt[:], start=True, stop=False)
                        nc.tensor.matmul(out=zps[:], lhsT=aggT[:], rhs=wt[:],
                                         start=False, stop=True)
                        # out = relu(z * s) with s = rs_dst (*rs_src for l<2)
                        z1 = sp.tile([128, fout], BF16 if li < 2 else F32,
                                     tag="z1", name=f"z1_{li}_{g}")
                        sv = rs_ds if li < 2 else rs_dst
                        nc.vector.tensor_scalar(
                            out=z1[:], in0=zps[:],
                            scalar1=sv[:, g:g + 1], scalar2=0.0,
                            op0=mybir.AluOpType.mult,
                            op1=mybir.AluOpType.max)
                        if li < 2:
                            nc.sync.dma_start(
                                hs[li + 1][g * 128:(g + 1) * 128, :], z1[:])
                        else:
                            nc.sync.dma_start(
                                out_t[g * 128:(g + 1) * 128, :], z1[:])
                    # fire AG halves for the next layer as groups complete
                    if li < 2:
                        done = (sg[-1] + 1) * 128
                        if done == AGH[1]:
                            ag_half(li + 1, 0)
                        elif done == GP:
                            ag_half(li + 1, 1)

    nc.compile()
    return nc


_cache = {}


def kernel(x, src, dst, W1, b1, W2, b2, W3, b3):
    global last_exec_time_ns
    x = np.asarray(x, np.float32)
    src_i = np.asarray(src)
    dst_i = np.asarray(dst)

    ckh = hash((src_i.tobytes(), dst_i.tobytes()))
    if ckh not in _cache:
        layout, nidx, nblk_total, gidx_cores, oh_cores = _prep_edges(
            src_i, dst_i)
        nc = _build(layout, nidx, nblk_total)
        _cache[ckh] = (nc, gidx_cores, oh_cores)
    nc, gidx_cores, oh_cores = _cache[ckh]

    deg_src = np.bincount(src_i.astype(np.int64), minlength=N).astype(np.float32)
    deg_dst = np.bincount(dst_i.astype(np.int64), minlength=N).astype(np.float32)

    in_maps = []
    for c in range(NC):
        sl = slice(c * SHARD, (c + 1) * SHARD)
        dd = np.ones(GP, np.float32)
        dd[:SHARD] = np.maximum(deg_dst[sl], 1.0)
        in_maps.append({
            "xs_in": _to_pgrid(x[sl]),
            "gidx_in": gidx_cores[c],
            "oh_in": oh_cores[c],
            "degs_in": _to_pgrid(deg_src[sl, None], fill=1.0),
            "degd_in": _to_pgrid(deg_dst[sl, None], fill=1.0),
            "degdr_in": dd[None, :],
            "w1_in": np.asarray(W1, np.float32),
            "w2_in": np.asarray(W2, np.float32),
            "w3_in": np.asarray(W3, np.float32),
            "b1_in": np.asarray(b1, np.float32)[None, :],
            "b2_in": np.asarray(b2, np.float32)[None, :],
            "b3_in": np.asarray(b3, np.float32)[None, :],
        })

    trace = bool(int(os.environ.get("GCN_TRACE", "0")))
    res = run_bass_kernel_spmd(nc, in_maps, core_ids=list(range(NC)),
                               trace=trace)
    last_exec_time_ns = res.exec_time_ns

    out = np.empty((N, DOUT), np.float32)
    for c in range(NC):
        out[c * SHARD:(c + 1) * SHARD] = res.results[c]["out_t"][:SHARD]
    return out
